# revision 58
# baseline (speedup 1.0000x reference)
"""HSIC loss kernel for 8 TRN2 NeuronCores.

Math: loss = -tr(CKW.CKG)/(n-1)^2 with CKX = KX.H, H = I - 1/n.
Expanded:  T = S1 - (2/n) sum_i sW_i sG_i + SW SG/n^2, loss = -T/(n-1)^2
where S1 = sum_ij KW.KG, sX = row sums of KX (KX symmetric).

Symmetry: only the region R = {(i,j): j >= 512*floor(i/512)} of each 4096^2
kernel block is computed (144 [128,512] tiles per matrix instead of 256).
For elements below R, the mirror (strictly-upper 512-blocks) supplies them:
S1 doubles those tiles' contributions, and row sums get the mirrored part
from COLUMN sums of the computed tiles (ones-vector matmuls into PSUM).

Sharding: [128,512]-tile-rows r=0..31; core c owns rows {2c,2c+1,30-2c,31-2c}
= 18 (r,chunk) pairs/core, a perfectly balanced split. The SPMD program is
IDENTICAL on every core: 18 uniform steps; all per-core variation is data
(lhsT/rhs/aug streams staged in compute order by the host).

Per step (one (r,jc) pair, W and G halves side by side in PSUM [128,1024]):
fp8(e4m3) DoubleRow matmuls: per half one aug matmul (K=12: 4-way fp8 splits
of a/2 = -sq/4 applied via 2.0-selector rows, covering both a_j columns and
a_i rows) then 2 DR matmuls (K=256 each) accumulate the dot products. ACT
does one pair-wide Exp -> tau (f16). DVE: fused custom POWSUM5 writes
k = f16(t+t^2+t^4+t^8+t^16) in a single rounding, then per half a 4x-mode
tensor_scalar reduce takes rowsum(k) (reading the SAME rounded k every other
consumer uses -- consistency is what makes per-entry f16 bias cancel under
the H-centering). S1 partials accumulate on the PE as diag(kW_q^T kG_q)
matmuls into two PSUM accumulators (upper steps 0-13 weight 2, straddle
steps 14-17 weight 1), each extracted via an identity-mask STT. Column
sums: ones[128,1] matmuls into per-group 32-aligned PSUM rows, flushed by 3
ACT copies + DMAs. cs/S1 matmul consumption runs LAG steps behind
production so no engine stalls on the cross-engine chain. Host combines in
f64 and replaces the (quantized) diagonal with its exact value of 5.
"""
import numpy as np
import ml_dtypes
from contextlib import ExitStack

import concourse.bass as bass
import concourse.tile as tile
from concourse import bacc, mybir
import concourse.dve_ops as dve_ops
from concourse.dve_spec import Spec, Src0, lower, _has_src1
from concourse.dve_ops import DveOp
from concourse.dve_uop import DveOpSpec

N_ROWS = 4096
TUNE_LAG = 4
TUNE_KPP = 6
TUNE_TAUP = 3
TUNE_DMP = 4
ABLATE = set()
FLUSH_POOL = False
ACT_RED = frozenset()
D = 512
NCORES = 8
P = 128
NSTEP = 18
NG = 9
F8 = ml_dtypes.float8_e4m3
LAST_RESULT = None
LAST_SCALE = None

f32 = mybir.dt.float32
f16 = mybir.dt.float16
f8e4 = mybir.dt.float8e4
DR = mybir.MatmulPerfMode.DoubleRow
ADD = mybir.AluOpType.add
MULT = mybir.AluOpType.mult


def _ref_powsum5(in0, in1, s0, s1, imm2):
    t = in0.astype(np.float32)
    t2 = t * t
    t4 = t2 * t2
    t8 = t4 * t4
    return (((t + t2) + (t4 + t8)) + t8 * t8).astype(np.float32)


def _register_powsum5():
    name = "POWSUM5_HSIC_ANT"
    for op in dve_ops.OPS:
        if op.name == name:
            return op
    t = Src0
    t2 = t * t
    t4 = t2 * t2
    t8 = t4 * t4
    spec = Spec(body=((t + t2) + (t4 + t8)) + t8 * t8, reference=_ref_powsum5)
    shas = {}
    for ver in ("v3", "v4"):
        tmp = DveOpSpec(name=name, opcode=1, uops=lower(spec, ver=ver),
                        rd1_en=_has_src1(spec))
        shas[ver] = tmp.sha(ver)
    op = DveOp(name, spec, subdim=False, uops_sha=shas)
    dve_ops.OPS.append(op)
    dve_ops._SUB_OPCODE_FOR_NAME[name] = (
        dve_ops._CUSTOM_DVE_ROW_BASE + len(dve_ops.OPS) - 1)
    dve_ops.CUSTOM_DVE_SPECS[name] = op.spec
    return op


def _schedule(c):
    """18 (tile_row, chunk, strict_upper) steps for core c. The 4 straddle
    pairs (jc == block row: counted once, no colsum mirror) come LAST as
    groups 7-8, after the 14 strict-upper pairs (chunk-major, groups 0-6),
    so the final cs flush+DMA chain clears the tail.
    This fixed straddle/upper step layout is identical on every core, so the
    two S1 PSUM accumulators can be routed by step index in the uniform
    SPMD program. Groups (consecutive step pairs) always share the chunk."""
    rows = [2 * c, 2 * c + 1, 30 - 2 * c, 31 - 2 * c]
    straddle = []
    upper = []
    for jc in range(8):
        for r in rows:
            if jc == r // 4:
                straddle.append((r, jc, False))
            elif jc > r // 4:
                upper.append((r, jc, True))
    steps = upper + straddle
    assert len(straddle) == 4 and len(steps) == NSTEP
    for g in range(NG):
        assert steps[2 * g][1] == steps[2 * g + 1][1]
        assert steps[2 * g][2] == steps[2 * g + 1][2]
    return steps


def _build(scale: float):
    POWSUM5 = _register_powsum5()
    nc = bacc.Bacc("TRN2", target_bir_lowering=False, debug=False)

    rwg_d = nc.dram_tensor("rwg", [P, NG * 4096], f8e4, kind="ExternalInput")
    lwg_d = nc.dram_tensor("lwg", [P, NG * 2048], f8e4, kind="ExternalInput")
    arl_d = nc.dram_tensor("arl", [12, NG * 2560], f8e4, kind="ExternalInput")
    id_d = nc.dram_tensor("ident", [P, 128], f16, kind="ExternalInput")
    acc_d = nc.dram_tensor("acc", [P, 2 * NSTEP + 2], f32, kind="ExternalOutput")
    cs_d = nc.dram_tensor("cs", [12, 1024], f32, kind="ExternalOutput")

    with tile.TileContext(nc) as tc, ExitStack() as ctx:
        const = ctx.enter_context(tc.tile_pool(name="const", bufs=1))
        psum = ctx.enter_context(tc.tile_pool(name="psum", bufs=2, space="PSUM"))
        csp = ctx.enter_context(tc.tile_pool(name="csp", bufs=1, space="PSUM"))
        taup = ctx.enter_context(tc.tile_pool(name="taup", bufs=TUNE_TAUP))
        kpp = ctx.enter_context(tc.tile_pool(name="kpp", bufs=TUNE_KPP))
        dmp = ctx.enter_context(tc.tile_pool(name="dmp", bufs=TUNE_DMP))

        rwg_t = const.tile([P, NG * 4096], f8e4, tag="rwg", name="rwg_t")
        lwg_t = const.tile([P, NG * 2048], f8e4, tag="lwg", name="lwg_t")
        arl_t = const.tile([12, NG * 2560], f8e4, tag="arl", name="arl_t")
        ones_t = const.tile([P, 1], f16, tag="ones", name="ones_t")
        acc_t = const.tile([P, 2 * NSTEP + 2], f32, tag="acc", name="acc_t")
        ident_t = const.tile([P, 128], f16, tag="ident", name="ident_t")
        stage = [const.tile([65, 1024], f32, tag=f"st{i}", name=f"st{i}")
                 for i in range(0 if "cs" in ABLATE else 3)]
        nc.vector.memset(ones_t[:], 1.0)
        # per-group prefetch in compute order: 3 combined DMAs per group
        for g in range(NG):
            if g == 2:
                nc.sync.dma_start(ident_t[:], id_d.ap()[:])
            if g == 0:
                nc.sync.dma_start(arl_t[:, 0:2560], arl_d.ap()[:, 0:2560])
                nc.sync.dma_start(rwg_t[:, 0:2048], rwg_d.ap()[:, 0:2048])
                nc.sync.dma_start(lwg_t[:, 0:2048], lwg_d.ap()[:, 0:2048])
                nc.sync.dma_start(rwg_t[:, 2048:4096],
                                  rwg_d.ap()[:, 2048:4096])
            else:
                nc.sync.dma_start(arl_t[:, g * 2560:(g + 1) * 2560],
                                  arl_d.ap()[:, g * 2560:(g + 1) * 2560])
                nc.sync.dma_start(lwg_t[:, g * 2048:(g + 1) * 2048],
                                  lwg_d.ap()[:, g * 2048:(g + 1) * 2048])
                nc.sync.dma_start(rwg_t[:, g * 4096:(g + 1) * 4096],
                                  rwg_d.ap()[:, g * 4096:(g + 1) * 4096])

        cs_tile = None if "cs" in ABLATE else csp.tile([65, 1024], f32, tag="cs0", name="cs0")
        s1_str = None if "s1" in ABLATE else csp.tile([P, 512], f32, tag="s1a", name="s1a")
        s1_upp = None if "s1" in ABLATE else csp.tile([P, 512], f32, tag="s1b", name="s1b")
        LAG = TUNE_LAG
        kp_list = {}
        flush_state = [0]
        pending_flush = []

        def emit_step(s):
            g, u = s // 2, s % 2
            ps = psum.tile([P, 1024], f32, tag="pair", name="pair")
            al_ap = arl_t[:, g * 2560 + 2048 + u * 256:
                          g * 2560 + 2048 + (u + 1) * 256].rearrange(
                "p (two m) -> p two m", two=2)
            for h in range(2):
                ar_ap = arl_t[:, g * 2560 + h * 1024:g * 2560 + (h + 1) * 1024] \
                    .rearrange("p (two n) -> p two n", two=2)
                nc.tensor.matmul(ps[:, h * 512:(h + 1) * 512], al_ap, ar_ap,
                                 start=True, stop=False, perf_mode=DR)
            for h in range(2):
                lbase = g * 2048 + h * 1024 + u * 512
                rbase = g * 4096 + h * 2048
                for kc in range(2):
                    lap = lwg_t[:, lbase + kc * 256:lbase + (kc + 1) * 256] \
                        .rearrange("p (two m) -> p two m", two=2)
                    rap = rwg_t[:, rbase + kc * 1024:rbase + (kc + 1) * 1024] \
                        .rearrange("p (two n) -> p two n", two=2)
                    nc.tensor.matmul(ps[:, h * 512:(h + 1) * 512], lap, rap,
                                     start=False, stop=(kc == 1), perf_mode=DR)
            tau = taup.tile([P, 1024], f16, tag="tau", name="tau")
            kp = kpp.tile([P, 1024], f16, tag="kp", name="kp")
            if s < 2:
                for h in range(2):
                    hs = slice(h * 512, (h + 1) * 512)
                    nc.scalar.activation(tau[:, hs], ps[:, hs],
                                         mybir.ActivationFunctionType.Exp,
                                         bias=0.0, scale=scale)
                    nc.vector._custom_dve(POWSUM5, out=kp[:, hs],
                                          in0=tau[:, hs])
            else:
                nc.scalar.activation(tau[:], ps[:],
                                     mybir.ActivationFunctionType.Exp,
                                     bias=0.0, scale=scale)
            while pending_flush:
                i, tile_ref = pending_flush.pop(0)
                if FLUSH_POOL:
                    nc.gpsimd.tensor_copy(stage[i][:], tile_ref[:])
                else:
                    nc.scalar.copy(stage[i][:], tile_ref[:])
                nc.sync.dma_start(cs_d.ap()[3 * i:3 * i + 3, :],
                                  stage[i][0:65:32, :])
            if s >= 2:
                nc.vector._custom_dve(POWSUM5, out=kp[:], in0=tau[:])
            for h in range(2):
                if h == 1 and s in ACT_RED:
                    continue
                sl = slice(h * 512, (h + 1) * 512)
                dummy = dmp.tile([P, 512], f16, tag="dm", name="dm")
                nc.vector.tensor_scalar(
                    out=dummy[:], in0=kp[:, sl], scalar1=1.0, scalar2=0.0,
                    op0=MULT, op1=ADD,
                    accum_out=acc_t[:, 2 * s + h:2 * s + h + 1])
            kp_list[s] = kp

        def emit_extract(i, accT):
            ddump = dmp.tile([P, 128], f32, tag="dd", name="dd")
            nc.vector.scalar_tensor_tensor(
                out=ddump[:], in0=accT[:, 0:128], scalar=1.0, in1=ident_t[:],
                op0=MULT, op1=MULT,
                accum_out=acc_t[:, 2 * NSTEP + i:2 * NSTEP + i + 1])

        def emit_lagged(s):
            g, u = s // 2, s % 2
            kp = kp_list.pop(s)
            if s in ACT_RED:
                dummy = dmp.tile([P, 512], f16, tag="dm", name="dm")
                nc.scalar.activation(dummy[:], kp[:, 512:1024],
                                     mybir.ActivationFunctionType.Copy,
                                     accum_out=acc_t[:, 2 * s + 1:2 * s + 2])
            # S1: accumulate kW_q^T . kG_q; its diagonal sums to sum(kW*kG).
            # Straddle steps (0-3) and strict-upper steps (4-17) use separate
            # accumulators (host weights them 1x / 2x).
            accT = s1_str if s >= 14 else s1_upp
            first = (s == 14) if s >= 14 else (s == 0)
            last = (s == NSTEP - 1) if s >= 14 else (s == 13)
            for q in range(4 if "s1" not in ABLATE else 0):
                nc.tensor.matmul(accT[:, 0:128],
                                 kp[:, q * 128:(q + 1) * 128],
                                 kp[:, 512 + q * 128:512 + (q + 1) * 128],
                                 start=(first and q == 0), stop=(last and q == 3),
                                 skip_group_check=True)
            # column sums into per-group PSUM row (32-aligned slot)
            if s < 14 and "cs" not in ABLATE:
                qrow = (g % 3) * 32
                for h in range(2):
                    nc.tensor.matmul(
                        cs_tile[qrow:qrow + 1, h * 512:(h + 1) * 512],
                        ones_t[:], kp[:, h * 512:(h + 1) * 512],
                        start=(u == 0), stop=(u == 1), skip_group_check=True)
            if u == 1 and g in (2, 5, 6) and "cs" not in ABLATE:
                pending_flush.append((flush_state[0], cs_tile))
                flush_state[0] += 1
            if s == 13 and "s1" not in ABLATE:
                emit_extract(1, s1_upp)

        for s in range(NSTEP):
            if s >= LAG:
                emit_lagged(s - LAG)
            emit_step(s)
        for s in range(NSTEP - LAG, NSTEP):
            emit_lagged(s)
        while pending_flush:
            i, tile_ref = pending_flush.pop(0)
            nc.scalar.copy(stage[i][:], tile_ref[:])
            nc.sync.dma_start(cs_d.ap()[3 * i:3 * i + 3, :],
                              stage[i][0:65:32, :])
        nc.sync.dma_start(acc_d.ap()[:, 0:2 * NSTEP - 4],
                          acc_t[:, 0:2 * NSTEP - 4])
        if "s1" not in ABLATE:
            emit_extract(0, s1_str)
        nc.sync.dma_start(acc_d.ap()[:, 2 * NSTEP - 4:],
                          acc_t[:, 2 * NSTEP - 4:])
    nc.compile()
    return nc


def _split4(x):
    """4-term fp8 split of x (f64): sum of returned rows ~ x."""
    outs = []
    r = x.copy()
    for _ in range(4):
        h = r.astype(F8)
        outs.append(h)
        r = r - h.astype(np.float64)
    return outs


def _k16_of_tau(tau16):
    """Device-replica: k16 = f16(powsum5_f32(f16 tau)); every consumer
    (rowsum reduce, S1 product, colsum matmul) reads this same value."""
    t = tau16.astype(np.float32)
    t2 = t * t
    t4 = t2 * t2
    t8 = t4 * t4
    k = (((t + t2) + (t4 + t8)) + t8 * t8).astype(np.float16)
    return k.astype(np.float64)


def kernel(W, G, **_):
    import os
    os.environ["BASS_NEVER_TRACE"] = "1"
    from concourse.bass_utils import run_bass_kernel_spmd
    W = np.asarray(W, dtype=np.float32)
    G = np.asarray(G, dtype=np.float32)
    n = W.shape[0]
    N = 2 * n

    # bandwidth from the full-precision inputs (closed form, f64)
    W64, G64 = W.astype(np.float64), G.astype(np.float64)
    sqW_t = (W64 * W64).sum(1)
    sqG_t = (G64 * G64).sum(1)
    colsum = W64.sum(0) + G64.sum(0)
    sum_d2 = 2.0 * N * (sqW_t.sum() + sqG_t.sum()) - 2.0 * (colsum * colsum).sum()
    bw = sum_d2 / (N * N - N) / 4.0
    scale = float(np.float32(1.0 / (8.0 * bw)))

    # fp8 quantization + aug splits (from quantized rows: keeps d2_q >= 0
    # and the diagonal exactly zero pre-rounding)
    W8 = W.astype(F8)
    G8 = G.astype(F8)
    W8f = W8.astype(np.float64)
    G8f = G8.astype(np.float64)
    aW = -0.5 * (W8f * W8f).sum(1)
    aG = -0.5 * (G8f * G8f).sum(1)
    # 4-term fp8 split of a/2 (e4m3 max is 240; |a| can exceed it), applied
    # through selector rows of 2.0 in the aug matmul.
    aW4 = _split4(aW / 2.0)
    aG4 = _split4(aG / 2.0)
    aWs = 2.0 * sum(a.astype(np.float64) for a in aW4)
    aGs = 2.0 * sum(a.astype(np.float64) for a in aG4)
    W8T = np.ascontiguousarray(W8.T)  # [feat, row]
    G8T = np.ascontiguousarray(G8.T)

    scheds = [_schedule(c) for c in range(NCORES)]
    in_maps = []
    for c in range(NCORES):
        st = scheds[c]
        rwg = np.zeros((P, NG * 4096), F8)
        lwg = np.zeros((P, NG * 2048), F8)
        arl = np.zeros((12, NG * 2560), F8)
        for g in range(NG):
            jc = st[2 * g][1]
            cols = slice(jc * 512, (jc + 1) * 512)
            for q in range(4):
                rwg[:, g * 4096 + q * 512:g * 4096 + (q + 1) * 512] = \
                    W8T[q * P:(q + 1) * P, cols]
                rwg[:, g * 4096 + 2048 + q * 512:g * 4096 + 2048 + (q + 1) * 512] = \
                    G8T[q * P:(q + 1) * P, cols]
                arl[q, g * 2560 + 0:g * 2560 + 512] = aW4[q][cols]
                arl[4 + q, g * 2560 + 0:g * 2560 + 512] = 2.0
                arl[q, g * 2560 + 1024:g * 2560 + 1536] = aG4[q][cols]
                arl[8 + q, g * 2560 + 1024:g * 2560 + 1536] = 2.0
        for s, (r, jc, su) in enumerate(st):
            g, u = s // 2, s % 2
            rsl = slice(r * P, (r + 1) * P)
            for kc in range(2):
                for i in range(2):
                    fsl = slice(kc * 256 + i * P, kc * 256 + (i + 1) * P)
                    base = g * 2048 + u * 512 + kc * 256 + i * P
                    lwg[:, base:base + P] = W8T[fsl, rsl]
                    lwg[:, 1024 + base:1024 + base + P] = G8T[fsl, rsl]
            abase = g * 2560 + 2048 + u * 256
            for q in range(4):
                arl[q, abase:abase + P] = 2.0
                arl[4 + q, abase:abase + P] = aW4[q][rsl]
                arl[8 + q, abase:abase + P] = aG4[q][rsl]
        in_maps.append({"rwg": rwg, "lwg": lwg, "arl": arl,
                        "ident": np.eye(P, dtype=np.float16)})

    global LAST_SCALE
    LAST_SCALE = scale
    nc = _build(scale)
    res = run_bass_kernel_spmd(nc, in_maps, core_ids=list(range(NCORES)))
    global LAST_RESULT
    LAST_RESULT = res

    # host combine (f64)
    S1 = 0.0
    sW = np.zeros(n)
    sG = np.zeros(n)
    for c in range(NCORES):
        out = res.results[c]
        acc = out["acc"].astype(np.float64)
        cs = out["cs"].astype(np.float64)
        for s, (r, jc, su) in enumerate(scheds[c]):
            rsl = slice(r * P, (r + 1) * P)
            sW[rsl] += acc[:, 2 * s + 0]
            sG[rsl] += acc[:, 2 * s + 1]
        S1 += acc[:, 2 * NSTEP].sum() + 2.0 * acc[:, 2 * NSTEP + 1].sum()
        for g in range(NG):
            jc, su = scheds[c][2 * g][1], scheds[c][2 * g][2]
            if su:
                csl = slice(jc * 512, (jc + 1) * 512)
                sW[csl] += cs[g, 0:512]
                sG[csl] += cs[g, 512:1024]

    # diagonal: replace device-computed quantized values with exact 5.0
    sc32 = np.float32(scale)
    for X8f, aXs, sX, which in ((W8f, aWs, sW, 0), (G8f, aGs, sG, 1)):
        g_ii = (X8f * X8f).sum(1)
        P_ii = (g_ii + 2.0 * aXs).astype(np.float32)
        tau = (np.exp(P_ii * sc32)).astype(np.float16)
        k16 = _k16_of_tau(tau)
        sX += 5.0 - k16
        if which == 0:
            kWd = k16
        else:
            kGd = k16
    S1 += (25.0 - kWd * kGd).sum()

    T = S1 - (2.0 / n) * (sW * sG).sum() + sW.sum() * sG.sum() / (n * n)
    loss = -T / ((n - 1) ** 2)
    return np.float32(loss)


# revision 59
# speedup vs baseline: 1.0052x; 1.0052x over previous
"""HSIC loss kernel for 8 TRN2 NeuronCores.

Math: loss = -tr(CKW.CKG)/(n-1)^2 with CKX = KX.H, H = I - 1/n.
Expanded:  T = S1 - (2/n) sum_i sW_i sG_i + SW SG/n^2, loss = -T/(n-1)^2
where S1 = sum_ij KW.KG, sX = row sums of KX (KX symmetric).

Symmetry: only the region R = {(i,j): j >= 512*floor(i/512)} of each 4096^2
kernel block is computed (144 [128,512] tiles per matrix instead of 256).
For elements below R, the mirror (strictly-upper 512-blocks) supplies them:
S1 doubles those tiles' contributions, and row sums get the mirrored part
from COLUMN sums of the computed tiles (ones-vector matmuls into PSUM).

Sharding: [128,512]-tile-rows r=0..31; core c owns rows {2c,2c+1,30-2c,31-2c}
= 18 (r,chunk) pairs/core, a perfectly balanced split. The SPMD program is
IDENTICAL on every core: 18 uniform steps; all per-core variation is data
(lhsT/rhs/aug streams staged in compute order by the host).

Per step (one (r,jc) pair, W and G halves side by side in PSUM [128,1024]):
fp8(e4m3) DoubleRow matmuls: per half one aug matmul (K=12: 4-way fp8 splits
of a/2 = -sq/4 applied via 2.0-selector rows, covering both a_j columns and
a_i rows) then 2 DR matmuls (K=256 each) accumulate the dot products. ACT
does one pair-wide Exp -> tau (f16). DVE: fused custom POWSUM5 writes
k = f16(t+t^2+t^4+t^8+t^16) in a single rounding, then per half a 4x-mode
tensor_scalar reduce takes rowsum(k) (reading the SAME rounded k every other
consumer uses -- consistency is what makes per-entry f16 bias cancel under
the H-centering). S1 partials accumulate on the PE as diag(kW_q^T kG_q)
matmuls into two PSUM accumulators (upper steps 0-13 weight 2, straddle
steps 14-17 weight 1), each extracted via an identity-mask STT. Column
sums: ones[128,1] matmuls into per-group 32-aligned PSUM rows, flushed by 3
ACT copies + DMAs. cs/S1 matmul consumption runs LAG steps behind
production so no engine stalls on the cross-engine chain. Host combines in
f64 and replaces the (quantized) diagonal with its exact value of 5.
"""
import numpy as np
import ml_dtypes
from contextlib import ExitStack

import concourse.bass as bass
import concourse.tile as tile
from concourse import bacc, mybir
import concourse.dve_ops as dve_ops
from concourse.dve_spec import Spec, Src0, lower, _has_src1
from concourse.dve_ops import DveOp
from concourse.dve_uop import DveOpSpec

N_ROWS = 4096
TUNE_LAG = 4
TUNE_KPP = 6
TUNE_TAUP = 3
TUNE_DMP = 4
ABLATE = set()
FLUSH_POOL = False
ACT_RED = frozenset()
D = 512
NCORES = 8
P = 128
NSTEP = 18
NG = 9
F8 = ml_dtypes.float8_e4m3
LAST_RESULT = None
LAST_SCALE = None

f32 = mybir.dt.float32
f16 = mybir.dt.float16
f8e4 = mybir.dt.float8e4
DR = mybir.MatmulPerfMode.DoubleRow
ADD = mybir.AluOpType.add
MULT = mybir.AluOpType.mult


def _ref_powsum5(in0, in1, s0, s1, imm2):
    t = in0.astype(np.float32)
    t2 = t * t
    t4 = t2 * t2
    t8 = t4 * t4
    return (((t + t2) + (t4 + t8)) + t8 * t8).astype(np.float32)


def _register_powsum5():
    name = "POWSUM5_HSIC_ANT"
    for op in dve_ops.OPS:
        if op.name == name:
            return op
    t = Src0
    t2 = t * t
    t4 = t2 * t2
    t8 = t4 * t4
    spec = Spec(body=((t + t2) + (t4 + t8)) + t8 * t8, reference=_ref_powsum5)
    shas = {}
    for ver in ("v3", "v4"):
        tmp = DveOpSpec(name=name, opcode=1, uops=lower(spec, ver=ver),
                        rd1_en=_has_src1(spec))
        shas[ver] = tmp.sha(ver)
    op = DveOp(name, spec, subdim=False, uops_sha=shas)
    dve_ops.OPS.append(op)
    dve_ops._SUB_OPCODE_FOR_NAME[name] = (
        dve_ops._CUSTOM_DVE_ROW_BASE + len(dve_ops.OPS) - 1)
    dve_ops.CUSTOM_DVE_SPECS[name] = op.spec
    return op


def _schedule(c):
    """18 (tile_row, chunk, strict_upper) steps for core c. The 4 straddle
    pairs (jc == block row: counted once, no colsum mirror) come LAST as
    groups 7-8, after the 14 strict-upper pairs (chunk-major, groups 0-6),
    so the final cs flush+DMA chain clears the tail.
    This fixed straddle/upper step layout is identical on every core, so the
    two S1 PSUM accumulators can be routed by step index in the uniform
    SPMD program. Groups (consecutive step pairs) always share the chunk."""
    rows = [2 * c, 2 * c + 1, 30 - 2 * c, 31 - 2 * c]
    straddle = []
    upper = []
    for jc in range(8):
        for r in rows:
            if jc == r // 4:
                straddle.append((r, jc, False))
            elif jc > r // 4:
                upper.append((r, jc, True))
    steps = upper + straddle
    assert len(straddle) == 4 and len(steps) == NSTEP
    for g in range(NG):
        assert steps[2 * g][1] == steps[2 * g + 1][1]
        assert steps[2 * g][2] == steps[2 * g + 1][2]
    return steps


def _build(scale: float):
    POWSUM5 = _register_powsum5()
    nc = bacc.Bacc("TRN2", target_bir_lowering=False, debug=False)

    rwg_d = nc.dram_tensor("rwg", [P, NG * 4096], f8e4, kind="ExternalInput")
    lwg_d = nc.dram_tensor("lwg", [P, NG * 2048], f8e4, kind="ExternalInput")
    arl_d = nc.dram_tensor("arl", [12, NG * 2560], f8e4, kind="ExternalInput")
    id_d = nc.dram_tensor("ident", [P, 128], f16, kind="ExternalInput")
    acc_d = nc.dram_tensor("acc", [P, 2 * NSTEP + 2], f32, kind="ExternalOutput")
    cs_d = nc.dram_tensor("cs", [12, 1024], f32, kind="ExternalOutput")

    with tile.TileContext(nc) as tc, ExitStack() as ctx:
        const = ctx.enter_context(tc.tile_pool(name="const", bufs=1))
        psum = ctx.enter_context(tc.tile_pool(name="psum", bufs=2, space="PSUM"))
        csp = ctx.enter_context(tc.tile_pool(name="csp", bufs=1, space="PSUM"))
        taup = ctx.enter_context(tc.tile_pool(name="taup", bufs=TUNE_TAUP))
        kpp = ctx.enter_context(tc.tile_pool(name="kpp", bufs=TUNE_KPP))
        dmp = ctx.enter_context(tc.tile_pool(name="dmp", bufs=TUNE_DMP))

        rwg_t = const.tile([P, NG * 4096], f8e4, tag="rwg", name="rwg_t")
        lwg_t = const.tile([P, NG * 2048], f8e4, tag="lwg", name="lwg_t")
        arl_t = const.tile([12, NG * 2560], f8e4, tag="arl", name="arl_t")
        ones_t = const.tile([P, 1], f16, tag="ones", name="ones_t")
        acc_t = const.tile([P, 2 * NSTEP + 2], f32, tag="acc", name="acc_t")
        ident_t = const.tile([P, 128], f16, tag="ident", name="ident_t")
        stage = [const.tile([65, 1024], f32, tag=f"st{i}", name=f"st{i}")
                 for i in range(0 if "cs" in ABLATE else 3)]
        nc.vector.memset(ones_t[:], 1.0)
        # per-group prefetch in compute order: 3 combined DMAs per group
        for g in range(NG):
            if g == 6:
                nc.sync.dma_start(ident_t[:], id_d.ap()[:])
            if g == 0:
                nc.sync.dma_start(arl_t[:, 0:2560], arl_d.ap()[:, 0:2560])
                nc.sync.dma_start(rwg_t[:, 0:2048], rwg_d.ap()[:, 0:2048])
                nc.sync.dma_start(lwg_t[:, 0:2048], lwg_d.ap()[:, 0:2048])
                nc.sync.dma_start(rwg_t[:, 2048:4096],
                                  rwg_d.ap()[:, 2048:4096])
            else:
                nc.sync.dma_start(arl_t[:, g * 2560:(g + 1) * 2560],
                                  arl_d.ap()[:, g * 2560:(g + 1) * 2560])
                nc.sync.dma_start(lwg_t[:, g * 2048:(g + 1) * 2048],
                                  lwg_d.ap()[:, g * 2048:(g + 1) * 2048])
                nc.sync.dma_start(rwg_t[:, g * 4096:(g + 1) * 4096],
                                  rwg_d.ap()[:, g * 4096:(g + 1) * 4096])

        cs_tile = None if "cs" in ABLATE else csp.tile([65, 1024], f32, tag="cs0", name="cs0")
        s1_str = None if "s1" in ABLATE else csp.tile([P, 512], f32, tag="s1a", name="s1a")
        s1_upp = None if "s1" in ABLATE else csp.tile([P, 512], f32, tag="s1b", name="s1b")
        LAG = TUNE_LAG
        kp_list = {}
        flush_state = [0]
        pending_flush = []

        def emit_step(s):
            g, u = s // 2, s % 2
            ps = psum.tile([P, 1024], f32, tag="pair", name="pair")
            al_ap = arl_t[:, g * 2560 + 2048 + u * 256:
                          g * 2560 + 2048 + (u + 1) * 256].rearrange(
                "p (two m) -> p two m", two=2)
            for h in range(2):
                ar_ap = arl_t[:, g * 2560 + h * 1024:g * 2560 + (h + 1) * 1024] \
                    .rearrange("p (two n) -> p two n", two=2)
                nc.tensor.matmul(ps[:, h * 512:(h + 1) * 512], al_ap, ar_ap,
                                 start=True, stop=False, perf_mode=DR)
            for h in range(2):
                lbase = g * 2048 + h * 1024 + u * 512
                rbase = g * 4096 + h * 2048
                for kc in range(2):
                    lap = lwg_t[:, lbase + kc * 256:lbase + (kc + 1) * 256] \
                        .rearrange("p (two m) -> p two m", two=2)
                    rap = rwg_t[:, rbase + kc * 1024:rbase + (kc + 1) * 1024] \
                        .rearrange("p (two n) -> p two n", two=2)
                    nc.tensor.matmul(ps[:, h * 512:(h + 1) * 512], lap, rap,
                                     start=False, stop=(kc == 1), perf_mode=DR)
            tau = taup.tile([P, 1024], f16, tag="tau", name="tau")
            kp = kpp.tile([P, 1024], f16, tag="kp", name="kp")
            if s < 2:
                for h in range(2):
                    hs = slice(h * 512, (h + 1) * 512)
                    nc.scalar.activation(tau[:, hs], ps[:, hs],
                                         mybir.ActivationFunctionType.Exp,
                                         bias=0.0, scale=scale)
                    nc.vector._custom_dve(POWSUM5, out=kp[:, hs],
                                          in0=tau[:, hs])
            else:
                nc.scalar.activation(tau[:], ps[:],
                                     mybir.ActivationFunctionType.Exp,
                                     bias=0.0, scale=scale)
            while pending_flush:
                i, tile_ref = pending_flush.pop(0)
                if FLUSH_POOL:
                    nc.gpsimd.tensor_copy(stage[i][:], tile_ref[:])
                else:
                    nc.scalar.copy(stage[i][:], tile_ref[:])
                nc.sync.dma_start(cs_d.ap()[3 * i:3 * i + 3, :],
                                  stage[i][0:65:32, :])
            if s >= 2:
                nc.vector._custom_dve(POWSUM5, out=kp[:], in0=tau[:])
            for h in range(2):
                if h == 1 and s in ACT_RED:
                    continue
                sl = slice(h * 512, (h + 1) * 512)
                dummy = dmp.tile([P, 512], f16, tag="dm", name="dm")
                nc.vector.tensor_scalar(
                    out=dummy[:], in0=kp[:, sl], scalar1=1.0, scalar2=0.0,
                    op0=MULT, op1=ADD,
                    accum_out=acc_t[:, 2 * s + h:2 * s + h + 1])
            kp_list[s] = kp

        def emit_extract(i, accT):
            ddump = dmp.tile([P, 128], f32, tag="dd", name="dd")
            nc.vector.scalar_tensor_tensor(
                out=ddump[:], in0=accT[:, 0:128], scalar=1.0, in1=ident_t[:],
                op0=MULT, op1=MULT,
                accum_out=acc_t[:, 2 * NSTEP + i:2 * NSTEP + i + 1])

        def emit_lagged(s):
            g, u = s // 2, s % 2
            kp = kp_list.pop(s)
            if s in ACT_RED:
                dummy = dmp.tile([P, 512], f16, tag="dm", name="dm")
                nc.scalar.activation(dummy[:], kp[:, 512:1024],
                                     mybir.ActivationFunctionType.Copy,
                                     accum_out=acc_t[:, 2 * s + 1:2 * s + 2])
            # S1: accumulate kW_q^T . kG_q; its diagonal sums to sum(kW*kG).
            # Straddle steps (0-3) and strict-upper steps (4-17) use separate
            # accumulators (host weights them 1x / 2x).
            accT = s1_str if s >= 14 else s1_upp
            first = (s == 14) if s >= 14 else (s == 0)
            last = (s == NSTEP - 1) if s >= 14 else (s == 13)
            for q in range(4 if "s1" not in ABLATE else 0):
                nc.tensor.matmul(accT[:, 0:128],
                                 kp[:, q * 128:(q + 1) * 128],
                                 kp[:, 512 + q * 128:512 + (q + 1) * 128],
                                 start=(first and q == 0), stop=(last and q == 3),
                                 skip_group_check=True)
            # column sums into per-group PSUM row (32-aligned slot)
            if s < 14 and "cs" not in ABLATE:
                qrow = (g % 3) * 32
                for h in range(2):
                    nc.tensor.matmul(
                        cs_tile[qrow:qrow + 1, h * 512:(h + 1) * 512],
                        ones_t[:], kp[:, h * 512:(h + 1) * 512],
                        start=(u == 0), stop=(u == 1), skip_group_check=True)
            if u == 1 and g in (2, 5, 6) and "cs" not in ABLATE:
                pending_flush.append((flush_state[0], cs_tile))
                flush_state[0] += 1
            if s == 13 and "s1" not in ABLATE:
                emit_extract(1, s1_upp)

        for s in range(NSTEP):
            if s >= LAG:
                emit_lagged(s - LAG)
            emit_step(s)
        for s in range(NSTEP - LAG, NSTEP):
            emit_lagged(s)
        while pending_flush:
            i, tile_ref = pending_flush.pop(0)
            nc.scalar.copy(stage[i][:], tile_ref[:])
            nc.sync.dma_start(cs_d.ap()[3 * i:3 * i + 3, :],
                              stage[i][0:65:32, :])
        nc.sync.dma_start(acc_d.ap()[:, 0:2 * NSTEP - 4],
                          acc_t[:, 0:2 * NSTEP - 4])
        if "s1" not in ABLATE:
            emit_extract(0, s1_str)
        nc.sync.dma_start(acc_d.ap()[:, 2 * NSTEP - 4:],
                          acc_t[:, 2 * NSTEP - 4:])
    nc.compile()
    return nc


def _split4(x):
    """4-term fp8 split of x (f64): sum of returned rows ~ x."""
    outs = []
    r = x.copy()
    for _ in range(4):
        h = r.astype(F8)
        outs.append(h)
        r = r - h.astype(np.float64)
    return outs


def _k16_of_tau(tau16):
    """Device-replica: k16 = f16(powsum5_f32(f16 tau)); every consumer
    (rowsum reduce, S1 product, colsum matmul) reads this same value."""
    t = tau16.astype(np.float32)
    t2 = t * t
    t4 = t2 * t2
    t8 = t4 * t4
    k = (((t + t2) + (t4 + t8)) + t8 * t8).astype(np.float16)
    return k.astype(np.float64)


def kernel(W, G, **_):
    import os
    os.environ["BASS_NEVER_TRACE"] = "1"
    from concourse.bass_utils import run_bass_kernel_spmd
    W = np.asarray(W, dtype=np.float32)
    G = np.asarray(G, dtype=np.float32)
    n = W.shape[0]
    N = 2 * n

    # bandwidth from the full-precision inputs (closed form, f64)
    W64, G64 = W.astype(np.float64), G.astype(np.float64)
    sqW_t = (W64 * W64).sum(1)
    sqG_t = (G64 * G64).sum(1)
    colsum = W64.sum(0) + G64.sum(0)
    sum_d2 = 2.0 * N * (sqW_t.sum() + sqG_t.sum()) - 2.0 * (colsum * colsum).sum()
    bw = sum_d2 / (N * N - N) / 4.0
    scale = float(np.float32(1.0 / (8.0 * bw)))

    # fp8 quantization + aug splits (from quantized rows: keeps d2_q >= 0
    # and the diagonal exactly zero pre-rounding)
    W8 = W.astype(F8)
    G8 = G.astype(F8)
    W8f = W8.astype(np.float64)
    G8f = G8.astype(np.float64)
    aW = -0.5 * (W8f * W8f).sum(1)
    aG = -0.5 * (G8f * G8f).sum(1)
    # 4-term fp8 split of a/2 (e4m3 max is 240; |a| can exceed it), applied
    # through selector rows of 2.0 in the aug matmul.
    aW4 = _split4(aW / 2.0)
    aG4 = _split4(aG / 2.0)
    aWs = 2.0 * sum(a.astype(np.float64) for a in aW4)
    aGs = 2.0 * sum(a.astype(np.float64) for a in aG4)
    W8T = np.ascontiguousarray(W8.T)  # [feat, row]
    G8T = np.ascontiguousarray(G8.T)

    scheds = [_schedule(c) for c in range(NCORES)]
    in_maps = []
    for c in range(NCORES):
        st = scheds[c]
        rwg = np.zeros((P, NG * 4096), F8)
        lwg = np.zeros((P, NG * 2048), F8)
        arl = np.zeros((12, NG * 2560), F8)
        for g in range(NG):
            jc = st[2 * g][1]
            cols = slice(jc * 512, (jc + 1) * 512)
            for q in range(4):
                rwg[:, g * 4096 + q * 512:g * 4096 + (q + 1) * 512] = \
                    W8T[q * P:(q + 1) * P, cols]
                rwg[:, g * 4096 + 2048 + q * 512:g * 4096 + 2048 + (q + 1) * 512] = \
                    G8T[q * P:(q + 1) * P, cols]
                arl[q, g * 2560 + 0:g * 2560 + 512] = aW4[q][cols]
                arl[4 + q, g * 2560 + 0:g * 2560 + 512] = 2.0
                arl[q, g * 2560 + 1024:g * 2560 + 1536] = aG4[q][cols]
                arl[8 + q, g * 2560 + 1024:g * 2560 + 1536] = 2.0
        for s, (r, jc, su) in enumerate(st):
            g, u = s // 2, s % 2
            rsl = slice(r * P, (r + 1) * P)
            for kc in range(2):
                for i in range(2):
                    fsl = slice(kc * 256 + i * P, kc * 256 + (i + 1) * P)
                    base = g * 2048 + u * 512 + kc * 256 + i * P
                    lwg[:, base:base + P] = W8T[fsl, rsl]
                    lwg[:, 1024 + base:1024 + base + P] = G8T[fsl, rsl]
            abase = g * 2560 + 2048 + u * 256
            for q in range(4):
                arl[q, abase:abase + P] = 2.0
                arl[4 + q, abase:abase + P] = aW4[q][rsl]
                arl[8 + q, abase:abase + P] = aG4[q][rsl]
        in_maps.append({"rwg": rwg, "lwg": lwg, "arl": arl,
                        "ident": np.eye(P, dtype=np.float16)})

    global LAST_SCALE
    LAST_SCALE = scale
    nc = _build(scale)
    res = run_bass_kernel_spmd(nc, in_maps, core_ids=list(range(NCORES)))
    global LAST_RESULT
    LAST_RESULT = res

    # host combine (f64)
    S1 = 0.0
    sW = np.zeros(n)
    sG = np.zeros(n)
    for c in range(NCORES):
        out = res.results[c]
        acc = out["acc"].astype(np.float64)
        cs = out["cs"].astype(np.float64)
        for s, (r, jc, su) in enumerate(scheds[c]):
            rsl = slice(r * P, (r + 1) * P)
            sW[rsl] += acc[:, 2 * s + 0]
            sG[rsl] += acc[:, 2 * s + 1]
        S1 += acc[:, 2 * NSTEP].sum() + 2.0 * acc[:, 2 * NSTEP + 1].sum()
        for g in range(NG):
            jc, su = scheds[c][2 * g][1], scheds[c][2 * g][2]
            if su:
                csl = slice(jc * 512, (jc + 1) * 512)
                sW[csl] += cs[g, 0:512]
                sG[csl] += cs[g, 512:1024]

    # diagonal: replace device-computed quantized values with exact 5.0
    sc32 = np.float32(scale)
    for X8f, aXs, sX, which in ((W8f, aWs, sW, 0), (G8f, aGs, sG, 1)):
        g_ii = (X8f * X8f).sum(1)
        P_ii = (g_ii + 2.0 * aXs).astype(np.float32)
        tau = (np.exp(P_ii * sc32)).astype(np.float16)
        k16 = _k16_of_tau(tau)
        sX += 5.0 - k16
        if which == 0:
            kWd = k16
        else:
            kGd = k16
    S1 += (25.0 - kWd * kGd).sum()

    T = S1 - (2.0 / n) * (sW * sG).sum() + sW.sum() * sG.sum() / (n * n)
    loss = -T / ((n - 1) ** 2)
    return np.float32(loss)
